# revision 19
# baseline (speedup 1.0000x reference)
"""Trainium2 Bass kernel for nn_Attention_13778255085887.

Dense multi-head attention block (EfficientViT-style):
  qkv 1x1 conv -> per-head softmax(q^T k * scale) -> v @ attn^T
  + depthwise conv(k=3) positional encoding on v -> proj 1x1 conv.

Shapes: B=8, dim=256, L=1024, heads=8, key_dim=16, head_dim=32.

Strategy: data-parallel over B across the 8 NeuronCores (zero collectives).
ScalarE (exp over 8.4M softmax elements/core, ~64us at 1 elem/lane/cycle) is
the bottleneck engine; the kernel is built so TensorE, VectorE, GpSimd and
DMA all hide underneath a back-to-back ScalarE exp chain:

  - q/k are projected into a packed layout (4 heads per 128-partition tile,
    head g at partitions 32g..32g+16, zero-padded to 32) so the tiny K=16
    score matmuls run 4-at-a-time via tile_position row groups.
  - S^T = k^T q is built per (head-pair, j-chunk) directly with j on
    partitions; softmax-without-max (logits provably in [-1.5, 1.5]) makes
    exp a single ScalarE ACTIVATE per (128, 1024) PSUM block, PSUM->SBUF
    with the 1/sqrt(d_k) scale folded in.
  - v^T (AV stationary operand) is computed directly as x^T @ w_v^T by
    matmul and v (natural layout, for the depthwise pe conv) by a second
    matmul -- no on-chip transposes anywhere.
  - AV out = (v^T)^T @ E accumulates over j in PSUM with 4 heads packed
    into one 128-partition tile via tile_position col groups; the softmax
    denominator accumulates in parallel via ones(128,32)^T @ E matmuls
    which also pre-broadcast d across each head's 32 output partitions.
  - y = av * (1/d) + pe(v) on VectorE (reciprocal_approx_fast), then the
    proj matmul with bias added on ScalarE/VectorE.

The phase-2 loop is software-pipelined: each step emits the NEXT step's
score quad before this step's AV/d quads so the PE stream stays one step
ahead of ScalarE; all remaining projection work ("extras") is drip-fed one
chunk per step with producer-before-consumer deadlines assert-checked.
PSUM budget (8 banks): 3 double-buffered S slots (6) + AV accumulator (1)
+ denominator accumulator (1); phase-1/3 matmuls borrow S slots.
"""

import os

import ml_dtypes
import numpy as np

import concourse.bass as bass
import concourse.mybir as mybir
import concourse.tile as tile
from concourse import bacc
from concourse.bass_utils import run_bass_kernel_spmd

BF16 = mybir.dt.bfloat16
F32 = mybir.dt.float32
AF = mybir.ActivationFunctionType
ALU = mybir.AluOpType

NH, KD, HD = 8, 16, 32
DIM, L, B = 256, 1024, 8
SCALE = KD ** -0.5  # 0.25


def _install_ntff_shim():
    """Optionally register the axon NTFF profiling hook (for trace=True).

    The container's antenv package lacks axon_hooks; recreate it and wire the
    ctypes-based hook from trn_agent_boot so neuron-profile exec times work.
    """
    import sys
    import types

    name = "antenv.axon_hooks"
    if name in sys.modules:
        return
    try:
        import antenv
        from trn_agent_boot.trn_boot import _ntff_profile_via_ctypes
    except ImportError:
        return
    hooks = types.ModuleType(name)
    hooks._the_hook = None
    hooks.set_axon_ntff_profile_hook = lambda h: setattr(hooks, "_the_hook", h)
    hooks.get_axon_ntff_profile_hook = lambda: hooks._the_hook
    sys.modules[name] = hooks
    antenv.axon_hooks = hooks
    so = "/opt/axon/libaxon_pjrt.so"
    if os.path.exists(so):
        hook = _ntff_profile_via_ctypes(so)
        if hook is not None:
            hooks.set_axon_ntff_profile_hook(hook)


def build_kernel() -> bass.Bass:
    nc = bacc.Bacc("TRN2", target_bir_lowering=False, debug=False, num_devices=8)

    # ---- DRAM I/O (per-core shard; weights replicated) ----
    x_d = nc.dram_tensor("x", (128, 2, 1024), BF16, kind="ExternalInput")
    wk_d = nc.dram_tensor("wk", (128, 2, 256), BF16, kind="ExternalInput")
    wq_d = nc.dram_tensor("wq", (128, 2, 256), BF16, kind="ExternalInput")
    wv_d = nc.dram_tensor("wv", (128, 2, 256), BF16, kind="ExternalInput")
    wpt_d = nc.dram_tensor("wpt", (128, 2, 256), BF16, kind="ExternalInput")
    bk_d = nc.dram_tensor("bk", (128, 2), F32, kind="ExternalInput")
    bq_d = nc.dram_tensor("bq", (128, 2), F32, kind="ExternalInput")
    bv_d = nc.dram_tensor("bv", (128, 2), F32, kind="ExternalInput")
    bvrow_d = nc.dram_tensor("bvrow", (1, 256), F32, kind="ExternalInput")
    wpe_d = nc.dram_tensor("wpe", (128, 2, 3), F32, kind="ExternalInput")
    bpe_d = nc.dram_tensor("bpe", (128, 2), F32, kind="ExternalInput")
    bproj_d = nc.dram_tensor("bproj", (128, 2), F32, kind="ExternalInput")
    onesb_d = nc.dram_tensor("onesb", (128, 1), BF16, kind="ExternalInput")
    onesf_d = nc.dram_tensor("onesf", (128, 32), F32, kind="ExternalInput")
    out_d = nc.dram_tensor("out", (128, 2, 1024), F32, kind="ExternalOutput")

    with tile.TileContext(nc) as tc:
        with (
            tc.tile_pool(name="const", bufs=1) as cpool,
            tc.tile_pool(name="work", bufs=4) as wpool,
            tc.tile_pool(name="epool", bufs=3) as epool,
            tc.tile_pool(name="ps_s", bufs=1, space="PSUM") as ps_s,
            tc.tile_pool(name="ps_av", bufs=1, space="PSUM") as ps_av,
            tc.tile_pool(name="ps_d", bufs=1, space="PSUM") as ps_d,
            tc.tile_pool(name="ps_misc", bufs=2, space="PSUM") as ps_misc,
        ):
            # ---- load constants / activations ----
            x_sb = cpool.tile([128, 2, 1024], BF16, tag="x")
            wk = cpool.tile([128, 2, 256], BF16, tag="wk")
            wq = cpool.tile([128, 2, 256], BF16, tag="wq")
            wv = cpool.tile([128, 2, 256], BF16, tag="wv")
            wpt = cpool.tile([128, 2, 256], BF16, tag="wpt")
            bk = cpool.tile([128, 2], F32, tag="bk")
            bq = cpool.tile([128, 2], F32, tag="bq")
            bv = cpool.tile([128, 2], F32, tag="bv")
            bvrow = cpool.tile([1, 256], F32, tag="bvrow")
            wpe = cpool.tile([128, 2, 3], F32, tag="wpe")
            bpe = cpool.tile([128, 2], F32, tag="bpe")
            bproj = cpool.tile([128, 2], F32, tag="bproj")
            onesb = cpool.tile([128, 1], BF16, tag="onesb")
            onesf = cpool.tile([128, 32], F32, tag="onesf")

            for sb, d in [
                (x_sb, x_d), (wk, wk_d), (wq, wq_d), (wv, wv_d), (wpt, wpt_d),
                (bk, bk_d), (bq, bq_d), (bv, bv_d), (bvrow, bvrow_d), (wpe, wpe_d),
                (bpe, bpe_d), (bproj, bproj_d), (onesb, onesb_d), (onesf, onesf_d),
            ]:
                nc.sync.dma_start(sb[:], d.ap())

            # persistent intermediates
            tk = cpool.tile([128, 2, 1024], BF16, tag="tk")        # packed k
            tq = cpool.tile([128, 2, 1024], BF16, tag="tq")        # packed q
            vnat = cpool.tile([128, 2, 1024], BF16, tag="vnat")    # v, natural
            vT = cpool.tile([128, 8, 256], BF16, tag="vT")         # v^T, j-chunked
            bvb = cpool.tile([128, 256], F32, tag="bvb")           # b_v row-broadcast
            peacc = cpool.tile([128, 2, 1024], BF16, tag="peacc")  # pe conv terms
            ybf = cpool.tile([128, 2, 1024], BF16, tag="ybf")      # y = av*R + pe
            zout = cpool.tile([128, 2, 1024], F32, tag="zout")

            # b_v broadcast across partitions for the v^T bias (free-dim bias)
            nc.gpsimd.partition_broadcast(bvb[:], bvrow[:], channels=128)

            # ---- phase-1 building blocks (emitted piecemeal) ----
            kq_ready = set()
            vt_ready = set()
            vn_ready = set()

            def emit_kq(t, n, w_sb, b_sb, dst):
                kq_ready.add((id(dst), t, n))
                ps = misc_ps()
                for kc in range(2):
                    nc.tensor.matmul(
                        ps[:], w_sb[:, kc, t * 128:(t + 1) * 128],
                        x_sb[:, kc, n * 512:(n + 1) * 512],
                        start=(kc == 0), stop=(kc == 1),
                    )
                nc.vector.tensor_scalar(
                    dst[:, t, n * 512:(n + 1) * 512], ps[:],
                    b_sb[:, t:t + 1], None, ALU.add,
                )

            def emit_vnat(t, n):
                vn_ready.add((t, n))
                ps = misc_ps()
                for kc in range(2):
                    nc.tensor.matmul(
                        ps[:], wv[:, kc, t * 128:(t + 1) * 128],
                        x_sb[:, kc, n * 512:(n + 1) * 512],
                        start=(kc == 0), stop=(kc == 1),
                    )
                nc.vector.tensor_scalar(
                    vnat[:, t, n * 512:(n + 1) * 512], ps[:],
                    bv[:, t:t + 1], None, ALU.add,
                )

            def emit_vt(jc):
                vt_ready.add(jc)
                ps = misc_ps()
                for kc in range(2):
                    nc.tensor.matmul(
                        ps[:, :256], x_sb[:, kc, jc * 128:(jc + 1) * 128],
                        wv[:, kc, :],
                        start=(kc == 0), stop=(kc == 1),
                    )
                nc.vector.tensor_tensor(vT[:, jc, :], ps[:, :256], bvb[:], ALU.add)

            # pe = depthwise conv(k=3, pad 1) on v + bias, into peacc[:, t, :]
            peacc_done = [False, False]
            pe_pending = []

            def emit_peacc(t):
                assert (t, 0) in vn_ready and (t, 1) in vn_ready
                nc.vector.tensor_scalar(
                    peacc[:, t, :], vnat[:, t, :], wpe[:, t, 1:2], bpe[:, t:t + 1],
                    ALU.mult, ALU.add,
                )
                tmp_l = wpool.tile([128, 1024], BF16, tag="pel", name=f"pel{t}")
                nc.vector.tensor_scalar(
                    tmp_l[:, :1023], vnat[:, t, :1023], wpe[:, t, 0:1], None,
                    ALU.mult,
                )
                nc.vector.tensor_tensor(
                    peacc[:, t, 1:], peacc[:, t, 1:], tmp_l[:, :1023], ALU.add,
                )
                tmp_r = wpool.tile([128, 1024], BF16, tag="per", name=f"per{t}")
                nc.vector.tensor_scalar(
                    tmp_r[:, :1023], vnat[:, t, 1:], wpe[:, t, 2:3], None,
                    ALU.mult,
                )
                nc.vector.tensor_tensor(
                    peacc[:, t, :1023], peacc[:, t, :1023], tmp_r[:, :1023], ALU.add,
                )
                peacc_done[t] = True
                for (tt_, nn_) in [p for p in pe_pending if p[0] == t]:
                    pe_pending.remove((tt_, nn_))
                    emit_pe_add(tt_, nn_)

            def emit_pe_add(t, n):
                nc.vector.tensor_tensor(
                    ybf[:, t, n * 512:(n + 1) * 512],
                    ybf[:, t, n * 512:(n + 1) * 512],
                    peacc[:, t, n * 512:(n + 1) * 512], ALU.add,
                )

            def emit_proj(mo, n, ps, bias_engine=None):
                for kc in range(2):
                    nc.tensor.matmul(
                        ps[:], wpt[:, kc, mo * 128:(mo + 1) * 128],
                        ybf[:, kc, n * 512:(n + 1) * 512],
                        start=(kc == 0), stop=(kc == 1),
                    )
                if bias_engine is nc.scalar:
                    nc.scalar.activation(
                        zout[:, mo, n * 512:(n + 1) * 512], ps[:],
                        AF.Identity, bias=bproj[:, mo:mo + 1],
                    )
                else:
                    nc.vector.tensor_scalar(
                        zout[:, mo, n * 512:(n + 1) * 512], ps[:],
                        bproj[:, mo:mo + 1], None, ALU.add,
                    )
                nc.sync.dma_start(
                    out_d.ap()[:, mo, n * 512:(n + 1) * 512],
                    zout[:, mo, n * 512:(n + 1) * 512],
                )

            # ---- phase 2: software-pipelined attention ----
            # steps (t, n, jc): per step one S quad / two exps / AV + d quads.
            # Phase-1 work not needed up front is drip-fed one chunk per step
            # ("extras") so ScalarE starts its exp chain as early as possible.
            steps = [
                (t, n, jc)
                for t in range(2) for n in range(2) for jc in range(8)
            ]
            av_tiles = {}
            d_tiles = {}

            def emit_s_step(step):
                # 4 score matmuls, one per row group -> run concurrently
                t, n, jc = step
                tiles = []
                assert (id(tk), t, 0) in kq_ready and (id(tq), t, n) in kq_ready
                assert jc < 4 or (id(tk), t, 1) in kq_ready
                for p in range(2):
                    s_ps = ps_s.tile(
                        [128, 1024], F32, tag="S", name=f"s_{t}_{n}_{jc}_{p}"
                    )
                    for gg in range(2):
                        g = 2 * p + gg
                        nc.tensor.matmul(
                            s_ps[:, gg * 512:(gg + 1) * 512],
                            tk[32 * g:32 * g + 16, t, jc * 128:(jc + 1) * 128],
                            tq[32 * g:32 * g + 16, t, n * 512:(n + 1) * 512],
                            start=True, stop=True,
                            tile_position=(32 * g, 0),
                        )
                    tiles.append(s_ps)
                return tiles

            def finish_tn(t, n):
                # R = 1/d (d ~ L, far from reciprocal_approx edge cases)
                av_ps = av_tiles.pop((t, n))
                d_ps = d_tiles.pop((t, n))
                rb_sb = wpool.tile([128, 512], F32, tag="rb")
                nc.vector.reciprocal_approx_fast(rb_sb[:], d_ps[:])
                nc.vector.tensor_tensor(
                    ybf[:, t, n * 512:(n + 1) * 512], av_ps[:], rb_sb[:],
                    ALU.mult,
                )
                if peacc_done[t]:
                    emit_pe_add(t, n)
                else:
                    pe_pending.append((t, n))

            # minimal front: q/k + first v^T chunk for step (0, 0, 0)
            emit_kq(0, 0, wk, bk, tk)
            emit_kq(0, 0, wq, bq, tq)
            emit_vt(0)
            # drip-fed producers; ORDER MATTERS: vT[jc] must be emitted no
            # later than step jc (its AV consumer), tk/tq halves before the
            # S quads that read them. Steps 1-2 pop two extras to make the
            # deadlines; readiness is assert-checked at the consumer sites.
            extras = {
                1: [lambda: emit_vt(1), lambda: emit_kq(0, 1, wk, bk, tk)],
                2: [lambda: emit_vt(2)],
                3: [lambda: emit_vt(3)],
                4: [lambda: emit_vt(4), lambda: emit_kq(0, 1, wq, bq, tq)],
                5: [lambda: emit_vt(5)],
                6: [lambda: emit_vt(6)],
                7: [lambda: emit_vt(7)],
                8: [lambda: emit_kq(1, 0, wk, bk, tk)],
                9: [lambda: emit_kq(1, 1, wk, bk, tk)],
                10: [lambda: emit_kq(1, 0, wq, bq, tq)],
                11: [lambda: emit_kq(1, 1, wq, bq, tq)],
                12: [lambda: emit_vnat(0, 0)],
                13: [lambda: emit_vnat(0, 1)],
                14: [lambda: emit_peacc(0)],
                15: [lambda: emit_vnat(1, 0)],
                16: [lambda: emit_vnat(1, 1)],
                17: [lambda: emit_peacc(1)],
                # n=0 proj units: both n=0 halves of y are ready after step
                # 23; run them here, borrowing the d-pool PSUM slot (its
                # (1,1) accumulation quads simply queue behind)
                26: [lambda: emit_proj(0, 0, ps_d.tile(
                    [128, 512], F32, tag="d", name="prj00"))],
                27: [lambda: emit_proj(1, 0, ps_d.tile(
                    [128, 512], F32, tag="d", name="prj10"))],
            }

            s_next = emit_s_step(steps[0])
            for i, step in enumerate(steps):
                t, n, jc = step
                s_cur = s_next
                e_sb = []
                for p in range(2):
                    e = epool.tile([128, 1024], BF16, tag="E", name=f"e{i}_{p}")
                    nc.scalar.activation(e[:], s_cur[p][:], AF.Exp, scale=SCALE)
                    e_sb.append(e)
                if i + 1 < len(steps):
                    s_next = emit_s_step(steps[i + 1])
                for fn in extras.pop(i, []):
                    fn()
                if (t, n) not in av_tiles:
                    av_tiles[(t, n)] = ps_av.tile(
                        [128, 512], F32, tag="av", name=f"av_{t}_{n}"
                    )
                    d_tiles[(t, n)] = ps_d.tile(
                        [128, 512], F32, tag="d", name=f"d_{t}_{n}"
                    )
                av_ps = av_tiles[(t, n)]
                d_ps = d_tiles[(t, n)]
                assert jc in vt_ready, (t, n, jc)
                for g in range(4):
                    h = 4 * t + g
                    nc.tensor.matmul(
                        av_ps[32 * g:32 * g + 32, :],
                        vT[:, jc, 32 * h:32 * h + 32],
                        e_sb[g // 2][:, (g % 2) * 512:(g % 2 + 1) * 512],
                        start=(jc == 0), stop=(jc == 7),
                        tile_position=(0, 32 * g),
                        skip_group_check=True,
                    )
                for g in range(4):
                    # denominator, pre-broadcast: ones(128,32)^T @ E fills
                    # all 32 partitions of head g with d_h[i]
                    nc.tensor.matmul(
                        d_ps[32 * g:32 * g + 32, :],
                        onesb[:, :32],
                        e_sb[g // 2][:, (g % 2) * 512:(g % 2 + 1) * 512],
                        start=(jc == 0), stop=(jc == 7),
                        tile_position=(0, 32 * g),
                        skip_group_check=True,
                    )
                if jc == 7:
                    finish_tn(t, n)
            assert not extras

            # ---- phase 3: remaining (n=1) proj units ----
            # kc=0 (reads the long-finished t=0 half of y) is emitted first so
            # it executes off the critical tail chain; kc=1 + bias + DMA wait
            # only on the last pe add.
            prj_ps = []
            for mo in range(2):
                ps = misc_ps()
                prj_ps.append(ps)
                nc.tensor.matmul(
                    ps[:], wpt[:, 0, mo * 128:(mo + 1) * 128],
                    ybf[:, 0, 512:], start=True, stop=False,
                )
            for mo in range(2):
                ps = prj_ps[mo]
                nc.tensor.matmul(
                    ps[:], wpt[:, 1, mo * 128:(mo + 1) * 128],
                    ybf[:, 1, 512:], start=False, stop=True,
                )
                nc.scalar.activation(
                    zout[:, mo, 512:], ps[:],
                    AF.Identity, bias=bproj[:, mo:mo + 1],
                )
                nc.sync.dma_start(
                    out_d.ap()[:, mo, 512:], zout[:, mo, 512:],
                )

    nc.compile()
    return nc


def pack_inputs(x, w_qkv, b_qkv, w_pe, b_pe, w_proj, b_proj):
    """Host-side packing of the full inputs into per-core in_maps."""
    bf16 = ml_dtypes.bfloat16
    f32 = np.float32

    # k/q packed layouts: tile t in {0,1}; partition m = 32*g + r; head h = 4t+g.
    # Only r < 16 is live (k channel r -> qkv row 64h+16+r; q channel r -> 64h+r);
    # r >= 16 columns are zero so both tiles stay 32-aligned per head.
    w_kA = np.zeros((256, 256), dtype=w_qkv.dtype)
    w_qA = np.zeros((256, 256), dtype=w_qkv.dtype)
    b_kP = np.zeros((128, 2), dtype=b_qkv.dtype)
    b_qP = np.zeros((128, 2), dtype=b_qkv.dtype)
    for t in range(2):
        for m in range(128):
            g, r = m // 32, m % 32
            h = 4 * t + g
            if r < 16:
                w_kA[:, t * 128 + m] = w_qkv[64 * h + 16 + r]
                w_qA[:, t * 128 + m] = w_qkv[64 * h + r]
                b_kP[m, t] = b_qkv[64 * h + 16 + r]
                b_qP[m, t] = b_qkv[64 * h + r]

    v_rows = np.array([64 * (c // 32) + 32 + c % 32 for c in range(256)])
    w_v = w_qkv[v_rows].T  # (256 d, 256 c)
    b_v = b_qkv[v_rows]

    def kpart(a):  # (256, F) -> (128, 2, F)
        return np.ascontiguousarray(a.reshape(2, 128, -1).transpose(1, 0, 2))

    def chan2(a):  # (256,) -> (128, 2)
        return np.ascontiguousarray(a.reshape(2, 128).T)

    common = {
        "wk": kpart(w_kA).astype(bf16),
        "wq": kpart(w_qA).astype(bf16),
        "wv": kpart(w_v).astype(bf16),
        "wpt": kpart(w_proj.T).astype(bf16),
        "bk": b_kP.astype(f32),
        "bq": b_qP.astype(f32),
        "bv": chan2(b_v).astype(f32),
        "bvrow": np.ascontiguousarray(b_v[None, :]).astype(f32),
        "wpe": kpart(w_pe[:, 0, :]).astype(f32),
        "bpe": chan2(b_pe).astype(f32),
        "bproj": chan2(b_proj).astype(f32),
        "onesb": np.ones((128, 1), dtype=bf16),
        "onesf": np.ones((128, 32), dtype=f32),
    }
    in_maps = []
    for b in range(B):
        m = dict(common)
        m["x"] = kpart(x[b]).astype(bf16)
        in_maps.append(m)
    return in_maps


_CACHE = {}


def kernel(x, w_qkv, b_qkv, w_pe, b_pe, w_proj, b_proj):
    x = np.asarray(x, dtype=np.float32)
    w_qkv = np.asarray(w_qkv, dtype=np.float32)
    b_qkv = np.asarray(b_qkv, dtype=np.float32)
    w_pe = np.asarray(w_pe, dtype=np.float32)
    b_pe = np.asarray(b_pe, dtype=np.float32)
    w_proj = np.asarray(w_proj, dtype=np.float32)
    b_proj = np.asarray(b_proj, dtype=np.float32)

    if "nc" not in _CACHE:
        _CACHE["nc"] = build_kernel()
    nc = _CACHE["nc"]

    in_maps = pack_inputs(x, w_qkv, b_qkv, w_pe, b_pe, w_proj, b_proj)

    trace = os.environ.get("BASS_KERNEL_TRACE", "") == "1"
    if trace:
        _install_ntff_shim()
    res = run_bass_kernel_spmd(
        nc, in_maps, core_ids=list(range(B)), trace=trace,
    )
    if trace:
        _CACHE["last_result"] = res

    out = np.empty((B, DIM, L), dtype=np.float32)
    for b in range(B):
        z = res.results[b]["out"]  # (128, 2, 1024)
        out[b] = z.transpose(1, 0, 2).reshape(DIM, L)
    return out


# revision 21
# speedup vs baseline: 1.0384x; 1.0384x over previous
"""Trainium2 Bass kernel for nn_Attention_13778255085887.

Dense multi-head attention block (EfficientViT-style):
  qkv 1x1 conv -> per-head softmax(q^T k * scale) -> v @ attn^T
  + depthwise conv(k=3) positional encoding on v -> proj 1x1 conv.

Shapes: B=8, dim=256, L=1024, heads=8, key_dim=16, head_dim=32.

Strategy: data-parallel over B across the 8 NeuronCores (zero collectives).
ScalarE (exp over 8.4M softmax elements/core, ~64us at 1 elem/lane/cycle) is
the bottleneck engine; the kernel is built so TensorE, VectorE, GpSimd and
DMA all hide underneath a back-to-back ScalarE exp chain:

  - q/k are projected into a packed layout (4 heads per 128-partition tile,
    head g at partitions 32g..32g+16, zero-padded to 32) so the tiny K=16
    score matmuls run 4-at-a-time via tile_position row groups.
  - S^T = k^T q is built per (head-pair, j-chunk) directly with j on
    partitions; softmax-without-max (logits provably in [-1.5, 1.5]) makes
    exp a single ScalarE ACTIVATE per (128, 1024) PSUM block, PSUM->SBUF
    with the 1/sqrt(d_k) scale folded in.
  - v^T (AV stationary operand) is computed directly as x^T @ w_v^T by
    matmul and v (natural layout, for the depthwise pe conv) by a second
    matmul -- no on-chip transposes anywhere.
  - AV out = (v^T)^T @ E accumulates over j in PSUM with 4 heads packed
    into one 128-partition tile via tile_position col groups; the softmax
    denominator accumulates in parallel via ones(128,32)^T @ E matmuls
    which also pre-broadcast d across each head's 32 output partitions.
  - y = av * (1/d) + pe(v) on VectorE (reciprocal_approx_fast), then the
    proj matmul with bias added on ScalarE/VectorE.

The phase-2 loop is software-pipelined: each step emits the NEXT step's
score quad before this step's AV/d quads so the PE stream stays one step
ahead of ScalarE; all remaining projection work ("extras") is drip-fed one
chunk per step with producer-before-consumer deadlines assert-checked.
PSUM budget (8 banks): 3 double-buffered S slots (6) + AV accumulator (1)
+ denominator accumulator (1); phase-1/3 matmuls borrow S slots.
"""

import os

import ml_dtypes
import numpy as np

import concourse.bass as bass
import concourse.mybir as mybir
import concourse.tile as tile
from concourse import bacc
from concourse.bass_utils import run_bass_kernel_spmd

BF16 = mybir.dt.bfloat16
F32 = mybir.dt.float32
AF = mybir.ActivationFunctionType
ALU = mybir.AluOpType

NH, KD, HD = 8, 16, 32
DIM, L, B = 256, 1024, 8
SCALE = KD ** -0.5  # 0.25


def _install_ntff_shim():
    """Optionally register the axon NTFF profiling hook (for trace=True).

    The container's antenv package lacks axon_hooks; recreate it and wire the
    ctypes-based hook from trn_agent_boot so neuron-profile exec times work.
    """
    import sys
    import types

    name = "antenv.axon_hooks"
    if name in sys.modules:
        return
    try:
        import antenv
        from trn_agent_boot.trn_boot import _ntff_profile_via_ctypes
    except ImportError:
        return
    hooks = types.ModuleType(name)
    hooks._the_hook = None
    hooks.set_axon_ntff_profile_hook = lambda h: setattr(hooks, "_the_hook", h)
    hooks.get_axon_ntff_profile_hook = lambda: hooks._the_hook
    sys.modules[name] = hooks
    antenv.axon_hooks = hooks
    so = "/opt/axon/libaxon_pjrt.so"
    if os.path.exists(so):
        hook = _ntff_profile_via_ctypes(so)
        if hook is not None:
            hooks.set_axon_ntff_profile_hook(hook)


def build_kernel() -> bass.Bass:
    nc = bacc.Bacc("TRN2", target_bir_lowering=False, debug=False, num_devices=8)

    # ---- DRAM I/O (per-core shard; weights replicated) ----
    x_d = nc.dram_tensor("x", (128, 2, 1024), BF16, kind="ExternalInput")
    wk_d = nc.dram_tensor("wk", (128, 2, 256), BF16, kind="ExternalInput")
    wq_d = nc.dram_tensor("wq", (128, 2, 256), BF16, kind="ExternalInput")
    wv_d = nc.dram_tensor("wv", (128, 2, 256), BF16, kind="ExternalInput")
    wpt_d = nc.dram_tensor("wpt", (128, 2, 256), BF16, kind="ExternalInput")
    bk_d = nc.dram_tensor("bk", (128, 2), F32, kind="ExternalInput")
    bq_d = nc.dram_tensor("bq", (128, 2), F32, kind="ExternalInput")
    bv_d = nc.dram_tensor("bv", (128, 2), F32, kind="ExternalInput")
    bvrow_d = nc.dram_tensor("bvrow", (1, 256), F32, kind="ExternalInput")
    wpe_d = nc.dram_tensor("wpe", (128, 2, 3), F32, kind="ExternalInput")
    bpe_d = nc.dram_tensor("bpe", (128, 2), F32, kind="ExternalInput")
    bproj_d = nc.dram_tensor("bproj", (128, 2), F32, kind="ExternalInput")
    onesb_d = nc.dram_tensor("onesb", (128, 1), BF16, kind="ExternalInput")
    onesf_d = nc.dram_tensor("onesf", (128, 32), F32, kind="ExternalInput")
    out_d = nc.dram_tensor("out", (128, 2, 1024), F32, kind="ExternalOutput")

    with tile.TileContext(nc) as tc:
        with (
            tc.tile_pool(name="const", bufs=1) as cpool,
            tc.tile_pool(name="work", bufs=4) as wpool,
            tc.tile_pool(name="epool", bufs=3) as epool,
            tc.tile_pool(name="ps_s", bufs=1, space="PSUM") as ps_s,
            tc.tile_pool(name="ps_av", bufs=1, space="PSUM") as ps_av,
            tc.tile_pool(name="ps_d", bufs=1, space="PSUM") as ps_d,
            tc.tile_pool(name="ps_misc", bufs=2, space="PSUM") as ps_misc,
        ):
            # ---- load constants / activations ----
            x_sb = cpool.tile([128, 2, 1024], BF16, tag="x")
            wk = cpool.tile([128, 2, 256], BF16, tag="wk")
            wq = cpool.tile([128, 2, 256], BF16, tag="wq")
            wv = cpool.tile([128, 2, 256], BF16, tag="wv")
            wpt = cpool.tile([128, 2, 256], BF16, tag="wpt")
            bk = cpool.tile([128, 2], F32, tag="bk")
            bq = cpool.tile([128, 2], F32, tag="bq")
            bv = cpool.tile([128, 2], F32, tag="bv")
            bvrow = cpool.tile([1, 256], F32, tag="bvrow")
            wpe = cpool.tile([128, 2, 3], F32, tag="wpe")
            bpe = cpool.tile([128, 2], F32, tag="bpe")
            bproj = cpool.tile([128, 2], F32, tag="bproj")
            onesb = cpool.tile([128, 1], BF16, tag="onesb")
            onesf = cpool.tile([128, 32], F32, tag="onesf")

            for sb, d in [
                (x_sb, x_d), (wk, wk_d), (wq, wq_d), (wv, wv_d), (wpt, wpt_d),
                (bk, bk_d), (bq, bq_d), (bv, bv_d), (bvrow, bvrow_d), (wpe, wpe_d),
                (bpe, bpe_d), (bproj, bproj_d), (onesb, onesb_d), (onesf, onesf_d),
            ]:
                nc.sync.dma_start(sb[:], d.ap())

            # persistent intermediates
            tk = cpool.tile([128, 2, 1024], BF16, tag="tk")        # packed k
            tq = cpool.tile([128, 2, 1024], BF16, tag="tq")        # packed q
            vnat = cpool.tile([128, 2, 1024], BF16, tag="vnat")    # v, natural
            vT = cpool.tile([128, 8, 256], BF16, tag="vT")         # v^T, j-chunked
            bvb = cpool.tile([128, 256], F32, tag="bvb")           # b_v row-broadcast
            peacc = cpool.tile([128, 2, 1024], BF16, tag="peacc")  # pe conv terms
            ybf = cpool.tile([128, 2, 1024], BF16, tag="ybf")      # y = av*R + pe
            zout = cpool.tile([128, 2, 1024], F32, tag="zout")

            # b_v broadcast across partitions for the v^T bias (free-dim bias)
            nc.gpsimd.partition_broadcast(bvb[:], bvrow[:], channels=128)

            # ---- phase-1 building blocks (emitted piecemeal) ----
            kq_ready = set()
            vt_ready = set()
            vn_ready = set()

            def emit_kq(t, n, w_sb, b_sb, dst):
                kq_ready.add((id(dst), t, n))
                ps = misc_ps()
                for kc in range(2):
                    nc.tensor.matmul(
                        ps[:], w_sb[:, kc, t * 128:(t + 1) * 128],
                        x_sb[:, kc, n * 512:(n + 1) * 512],
                        start=(kc == 0), stop=(kc == 1),
                    )
                nc.vector.tensor_scalar(
                    dst[:, t, n * 512:(n + 1) * 512], ps[:],
                    b_sb[:, t:t + 1], None, ALU.add,
                )

            def emit_vnat(t, n):
                vn_ready.add((t, n))
                ps = misc_ps()
                for kc in range(2):
                    nc.tensor.matmul(
                        ps[:], wv[:, kc, t * 128:(t + 1) * 128],
                        x_sb[:, kc, n * 512:(n + 1) * 512],
                        start=(kc == 0), stop=(kc == 1),
                    )
                nc.vector.tensor_scalar(
                    vnat[:, t, n * 512:(n + 1) * 512], ps[:],
                    bv[:, t:t + 1], None, ALU.add,
                )

            def emit_vt(jc):
                vt_ready.add(jc)
                ps = misc_ps()
                for kc in range(2):
                    nc.tensor.matmul(
                        ps[:, :256], x_sb[:, kc, jc * 128:(jc + 1) * 128],
                        wv[:, kc, :],
                        start=(kc == 0), stop=(kc == 1),
                    )
                nc.vector.tensor_tensor(vT[:, jc, :], ps[:, :256], bvb[:], ALU.add)

            # pe = depthwise conv(k=3, pad 1) on v + bias, into peacc[:, t, :]
            peacc_done = [False, False]
            pe_pending = []

            def emit_peacc(t):
                assert (t, 0) in vn_ready and (t, 1) in vn_ready
                nc.vector.tensor_scalar(
                    peacc[:, t, :], vnat[:, t, :], wpe[:, t, 1:2], bpe[:, t:t + 1],
                    ALU.mult, ALU.add,
                )
                tmp_l = wpool.tile([128, 1024], BF16, tag="pel", name=f"pel{t}")
                nc.vector.tensor_scalar(
                    tmp_l[:, :1023], vnat[:, t, :1023], wpe[:, t, 0:1], None,
                    ALU.mult,
                )
                nc.vector.tensor_tensor(
                    peacc[:, t, 1:], peacc[:, t, 1:], tmp_l[:, :1023], ALU.add,
                )
                tmp_r = wpool.tile([128, 1024], BF16, tag="per", name=f"per{t}")
                nc.vector.tensor_scalar(
                    tmp_r[:, :1023], vnat[:, t, 1:], wpe[:, t, 2:3], None,
                    ALU.mult,
                )
                nc.vector.tensor_tensor(
                    peacc[:, t, :1023], peacc[:, t, :1023], tmp_r[:, :1023], ALU.add,
                )
                peacc_done[t] = True
                for (tt_, nn_) in [p for p in pe_pending if p[0] == t]:
                    pe_pending.remove((tt_, nn_))
                    emit_pe_add(tt_, nn_)

            def emit_pe_add(t, n):
                nc.vector.tensor_tensor(
                    ybf[:, t, n * 512:(n + 1) * 512],
                    ybf[:, t, n * 512:(n + 1) * 512],
                    peacc[:, t, n * 512:(n + 1) * 512], ALU.add,
                )

            def emit_proj(mo, n, ps, bias_engine=None):
                for kc in range(2):
                    nc.tensor.matmul(
                        ps[:], wpt[:, kc, mo * 128:(mo + 1) * 128],
                        ybf[:, kc, n * 512:(n + 1) * 512],
                        start=(kc == 0), stop=(kc == 1),
                    )
                if bias_engine is nc.scalar:
                    nc.scalar.activation(
                        zout[:, mo, n * 512:(n + 1) * 512], ps[:],
                        AF.Identity, bias=bproj[:, mo:mo + 1],
                    )
                else:
                    nc.vector.tensor_scalar(
                        zout[:, mo, n * 512:(n + 1) * 512], ps[:],
                        bproj[:, mo:mo + 1], None, ALU.add,
                    )
                nc.sync.dma_start(
                    out_d.ap()[:, mo, n * 512:(n + 1) * 512],
                    zout[:, mo, n * 512:(n + 1) * 512],
                )

            # ---- phase 2: software-pipelined attention ----
            # steps (t, n, jc): per step one S quad / two exps / AV + d quads.
            # Phase-1 work not needed up front is drip-fed one chunk per step
            # ("extras") so ScalarE starts its exp chain as early as possible.
            steps = [
                (t, n, jc)
                for t in range(2) for n in range(2) for jc in range(8)
            ]
            av_tiles = {}
            d_tiles = {}

            def emit_s_step(step):
                # 4 score matmuls, one per row group -> run concurrently
                t, n, jc = step
                tiles = []
                assert (id(tk), t, 0) in kq_ready and (id(tq), t, n) in kq_ready
                assert jc < 4 or (id(tk), t, 1) in kq_ready
                for p in range(2):
                    s_ps = ps_s.tile(
                        [128, 1024], F32, tag="S", name=f"s_{t}_{n}_{jc}_{p}"
                    )
                    for gg in range(2):
                        g = 2 * p + gg
                        nc.tensor.matmul(
                            s_ps[:, gg * 512:(gg + 1) * 512],
                            tk[32 * g:32 * g + 16, t, jc * 128:(jc + 1) * 128],
                            tq[32 * g:32 * g + 16, t, n * 512:(n + 1) * 512],
                            start=True, stop=True,
                            tile_position=(32 * g, 0),
                        )
                    tiles.append(s_ps)
                return tiles

            def finish_tn(t, n):
                # R = 1/d (d ~ L, far from reciprocal_approx edge cases)
                av_ps = av_tiles.pop((t, n))
                d_ps = d_tiles.pop((t, n))
                rb_sb = wpool.tile([128, 512], F32, tag="rb")
                nc.vector.reciprocal_approx_fast(rb_sb[:], d_ps[:])
                nc.vector.tensor_tensor(
                    ybf[:, t, n * 512:(n + 1) * 512], av_ps[:], rb_sb[:],
                    ALU.mult,
                )
                if peacc_done[t]:
                    emit_pe_add(t, n)
                else:
                    pe_pending.append((t, n))

            # minimal front: q/k + first v^T chunk for step (0, 0, 0)
            emit_kq(0, 0, wk, bk, tk)
            emit_kq(0, 0, wq, bq, tq)
            emit_vt(0)
            # drip-fed producers; ORDER MATTERS: vT[jc] must be emitted no
            # later than step jc (its AV consumer), tk/tq halves before the
            # S quads that read them. Steps 1-2 pop two extras to make the
            # deadlines; readiness is assert-checked at the consumer sites.
            extras = {
                1: [lambda: emit_vt(1), lambda: emit_kq(0, 1, wk, bk, tk)],
                2: [lambda: emit_vt(2)],
                3: [lambda: emit_vt(3)],
                4: [lambda: emit_vt(4), lambda: emit_kq(0, 1, wq, bq, tq)],
                5: [lambda: emit_vt(5)],
                6: [lambda: emit_vt(6)],
                7: [lambda: emit_vt(7)],
                8: [lambda: emit_kq(1, 0, wk, bk, tk)],
                9: [lambda: emit_kq(1, 1, wk, bk, tk)],
                10: [lambda: emit_kq(1, 0, wq, bq, tq)],
                11: [lambda: emit_kq(1, 1, wq, bq, tq)],
                12: [lambda: emit_vnat(0, 0)],
                13: [lambda: emit_vnat(0, 1)],
                14: [lambda: emit_peacc(0)],
                15: [lambda: emit_vnat(1, 0)],
                16: [lambda: emit_vnat(1, 1)],
                17: [lambda: emit_peacc(1)],
                # n=0 proj units: both n=0 halves of y are ready after step
                # 23; run them here, borrowing the d-pool PSUM slot (its
                # (1,1) accumulation quads simply queue behind)
                26: [lambda: emit_proj(0, 0, ps_d.tile(
                    [128, 512], F32, tag="d", name="prj00"))],
                27: [lambda: emit_proj(1, 0, ps_d.tile(
                    [128, 512], F32, tag="d", name="prj10"))],
            }

            def emit_exp_dve(e_out, s_ps, tag):
                # exp(SCALE*x) via degree-4 Taylor on VectorE; logits are in
                # [-1.5, 1.5] so truncation error is ~1e-4 of the bf16 noise.
                # Offloads ~1us/block from the ScalarE critical chain.
                u = wpool.tile([128, 1024], BF16, tag="tay_u", name=f"u{tag}")
                nc.vector.tensor_scalar(u[:], s_ps[:], SCALE, None, ALU.mult)
                u2 = wpool.tile([128, 1024], BF16, tag="tay_u2", name=f"u2{tag}")
                nc.vector.tensor_tensor(u2[:], u[:], u[:], ALU.mult)
                h0 = wpool.tile([128, 1024], BF16, tag="tay_h", name=f"h{tag}")
                nc.vector.tensor_scalar(h0[:], u[:], 1.0 / 6.0, 0.5,
                                        ALU.mult, ALU.add)
                # h = y^2/24 + (1/2 + y/6)
                nc.vector.affine_then_add(h0[:], u2[:], h0[:], 1.0 / 24.0, 0.0)
                f = wpool.tile([128, 1024], BF16, tag="tay_f", name=f"f{tag}")
                nc.vector.tensor_tensor(f[:], u2[:], h0[:], ALU.mult)
                # e = (y + 1) + y^2 * h
                nc.vector.affine_then_add(e_out[:], u[:], f[:], 1.0, 1.0)

            # blocks whose exp runs on VectorE instead (steps where DVE is
            # otherwise idle -- after the extras, away from the tail chain)
            DVE_EXP = set()

            s_next = emit_s_step(steps[0])
            for i, step in enumerate(steps):
                t, n, jc = step
                s_cur = s_next
                e_sb = []
                for p in range(2):
                    e = epool.tile([128, 1024], BF16, tag="E", name=f"e{i}_{p}")
                    if i in DVE_EXP and p == 1:
                        emit_exp_dve(e, s_cur[p], f"{i}")
                    else:
                        nc.scalar.activation(e[:], s_cur[p][:], AF.Exp, scale=SCALE)
                    e_sb.append(e)
                if i + 1 < len(steps):
                    s_next = emit_s_step(steps[i + 1])
                for fn in extras.pop(i, []):
                    fn()
                if (t, n) not in av_tiles:
                    av_tiles[(t, n)] = ps_av.tile(
                        [128, 512], F32, tag="av", name=f"av_{t}_{n}"
                    )
                    d_tiles[(t, n)] = ps_d.tile(
                        [128, 512], F32, tag="d", name=f"d_{t}_{n}"
                    )
                av_ps = av_tiles[(t, n)]
                d_ps = d_tiles[(t, n)]
                assert jc in vt_ready, (t, n, jc)
                for g in range(4):
                    h = 4 * t + g
                    nc.tensor.matmul(
                        av_ps[32 * g:32 * g + 32, :],
                        vT[:, jc, 32 * h:32 * h + 32],
                        e_sb[g // 2][:, (g % 2) * 512:(g % 2 + 1) * 512],
                        start=(jc == 0), stop=(jc == 7),
                        tile_position=(0, 32 * g),
                        skip_group_check=True,
                    )
                for g in range(4):
                    # denominator, pre-broadcast: ones(128,32)^T @ E fills
                    # all 32 partitions of head g with d_h[i]
                    nc.tensor.matmul(
                        d_ps[32 * g:32 * g + 32, :],
                        onesb[:, :32],
                        e_sb[g // 2][:, (g % 2) * 512:(g % 2 + 1) * 512],
                        start=(jc == 0), stop=(jc == 7),
                        tile_position=(0, 32 * g),
                        skip_group_check=True,
                    )
                if jc == 7:
                    finish_tn(t, n)
            assert not extras

            # ---- phase 3: remaining (n=1) proj units ----
            # kc=0 (reads the long-finished t=0 half of y) is emitted first so
            # it executes off the critical tail chain; kc=1 + bias + DMA wait
            # only on the last pe add.
            prj_ps = []
            for mo in range(2):
                ps = misc_ps()
                prj_ps.append(ps)
                nc.tensor.matmul(
                    ps[:], wpt[:, 0, mo * 128:(mo + 1) * 128],
                    ybf[:, 0, 512:], start=True, stop=False,
                )
            for mo in range(2):
                ps = prj_ps[mo]
                nc.tensor.matmul(
                    ps[:], wpt[:, 1, mo * 128:(mo + 1) * 128],
                    ybf[:, 1, 512:], start=False, stop=True,
                )
                nc.scalar.activation(
                    zout[:, mo, 512:], ps[:],
                    AF.Identity, bias=bproj[:, mo:mo + 1],
                )
                nc.sync.dma_start(
                    out_d.ap()[:, mo, 512:], zout[:, mo, 512:],
                )

    nc.compile()
    return nc


def pack_inputs(x, w_qkv, b_qkv, w_pe, b_pe, w_proj, b_proj):
    """Host-side packing of the full inputs into per-core in_maps."""
    bf16 = ml_dtypes.bfloat16
    f32 = np.float32

    # k/q packed layouts: tile t in {0,1}; partition m = 32*g + r; head h = 4t+g.
    # Only r < 16 is live (k channel r -> qkv row 64h+16+r; q channel r -> 64h+r);
    # r >= 16 columns are zero so both tiles stay 32-aligned per head.
    w_kA = np.zeros((256, 256), dtype=w_qkv.dtype)
    w_qA = np.zeros((256, 256), dtype=w_qkv.dtype)
    b_kP = np.zeros((128, 2), dtype=b_qkv.dtype)
    b_qP = np.zeros((128, 2), dtype=b_qkv.dtype)
    for t in range(2):
        for m in range(128):
            g, r = m // 32, m % 32
            h = 4 * t + g
            if r < 16:
                w_kA[:, t * 128 + m] = w_qkv[64 * h + 16 + r]
                w_qA[:, t * 128 + m] = w_qkv[64 * h + r]
                b_kP[m, t] = b_qkv[64 * h + 16 + r]
                b_qP[m, t] = b_qkv[64 * h + r]

    v_rows = np.array([64 * (c // 32) + 32 + c % 32 for c in range(256)])
    w_v = w_qkv[v_rows].T  # (256 d, 256 c)
    b_v = b_qkv[v_rows]

    def kpart(a):  # (256, F) -> (128, 2, F)
        return np.ascontiguousarray(a.reshape(2, 128, -1).transpose(1, 0, 2))

    def chan2(a):  # (256,) -> (128, 2)
        return np.ascontiguousarray(a.reshape(2, 128).T)

    common = {
        "wk": kpart(w_kA).astype(bf16),
        "wq": kpart(w_qA).astype(bf16),
        "wv": kpart(w_v).astype(bf16),
        "wpt": kpart(w_proj.T).astype(bf16),
        "bk": b_kP.astype(f32),
        "bq": b_qP.astype(f32),
        "bv": chan2(b_v).astype(f32),
        "bvrow": np.ascontiguousarray(b_v[None, :]).astype(f32),
        "wpe": kpart(w_pe[:, 0, :]).astype(f32),
        "bpe": chan2(b_pe).astype(f32),
        "bproj": chan2(b_proj).astype(f32),
        "onesb": np.ones((128, 1), dtype=bf16),
        "onesf": np.ones((128, 32), dtype=f32),
    }
    in_maps = []
    for b in range(B):
        m = dict(common)
        m["x"] = kpart(x[b]).astype(bf16)
        in_maps.append(m)
    return in_maps


_CACHE = {}


def kernel(x, w_qkv, b_qkv, w_pe, b_pe, w_proj, b_proj):
    x = np.asarray(x, dtype=np.float32)
    w_qkv = np.asarray(w_qkv, dtype=np.float32)
    b_qkv = np.asarray(b_qkv, dtype=np.float32)
    w_pe = np.asarray(w_pe, dtype=np.float32)
    b_pe = np.asarray(b_pe, dtype=np.float32)
    w_proj = np.asarray(w_proj, dtype=np.float32)
    b_proj = np.asarray(b_proj, dtype=np.float32)

    if "nc" not in _CACHE:
        _CACHE["nc"] = build_kernel()
    nc = _CACHE["nc"]

    in_maps = pack_inputs(x, w_qkv, b_qkv, w_pe, b_pe, w_proj, b_proj)

    trace = os.environ.get("BASS_KERNEL_TRACE", "") == "1"
    if trace:
        _install_ntff_shim()
    res = run_bass_kernel_spmd(
        nc, in_maps, core_ids=list(range(B)), trace=trace,
    )
    if trace:
        _CACHE["last_result"] = res

    out = np.empty((B, DIM, L), dtype=np.float32)
    for b in range(B):
        z = res.results[b]["out"]  # (128, 2, 1024)
        out[b] = z.transpose(1, 0, 2).reshape(DIM, L)
    return out


# revision 22
# speedup vs baseline: 1.0443x; 1.0057x over previous
"""Trainium2 Bass kernel for nn_Attention_13778255085887.

Dense multi-head attention block (EfficientViT-style):
  qkv 1x1 conv -> per-head softmax(q^T k * scale) -> v @ attn^T
  + depthwise conv(k=3) positional encoding on v -> proj 1x1 conv.

Shapes: B=8, dim=256, L=1024, heads=8, key_dim=16, head_dim=32.

Strategy: data-parallel over B across the 8 NeuronCores (zero collectives).
ScalarE (exp over 8.4M softmax elements/core, ~64us at 1 elem/lane/cycle) is
the bottleneck engine; the kernel is built so TensorE, VectorE, GpSimd and
DMA all hide underneath a back-to-back ScalarE exp chain:

  - q/k are projected into a packed layout (4 heads per 128-partition tile,
    head g at partitions 32g..32g+16, zero-padded to 32) so the tiny K=16
    score matmuls run 4-at-a-time via tile_position row groups.
  - S^T = k^T q is built per (head-pair, j-chunk) directly with j on
    partitions; softmax-without-max (logits provably in [-1.5, 1.5]) makes
    exp a single ScalarE ACTIVATE per (128, 1024) PSUM block, PSUM->SBUF
    with the 1/sqrt(d_k) scale folded in.
  - v^T (AV stationary operand) is computed directly as x^T @ w_v^T by
    matmul and v (natural layout, for the depthwise pe conv) by a second
    matmul -- no on-chip transposes anywhere.
  - AV out = (v^T)^T @ E accumulates over j in PSUM with 4 heads packed
    into one 128-partition tile via tile_position col groups; the softmax
    denominator accumulates in parallel via ones(128,32)^T @ E matmuls
    which also pre-broadcast d across each head's 32 output partitions.
  - y = av * (1/d) + pe(v) on VectorE (reciprocal_approx_fast), then the
    proj matmul with bias added on ScalarE/VectorE.

The phase-2 loop is software-pipelined: each step emits the NEXT step's
score quad before this step's AV/d quads so the PE stream stays one step
ahead of ScalarE; all remaining projection work ("extras") is drip-fed one
chunk per step with producer-before-consumer deadlines assert-checked.
PSUM budget (8 banks): 3 double-buffered S slots (6) + AV accumulator (1)
+ denominator accumulator (1); phase-1/3 matmuls borrow S slots.
"""

import os

import ml_dtypes
import numpy as np

import concourse.bass as bass
import concourse.mybir as mybir
import concourse.tile as tile
from concourse import bacc
from concourse.bass_utils import run_bass_kernel_spmd

BF16 = mybir.dt.bfloat16
F32 = mybir.dt.float32
AF = mybir.ActivationFunctionType
ALU = mybir.AluOpType

NH, KD, HD = 8, 16, 32
DIM, L, B = 256, 1024, 8
SCALE = KD ** -0.5  # 0.25


def _install_ntff_shim():
    """Optionally register the axon NTFF profiling hook (for trace=True).

    The container's antenv package lacks axon_hooks; recreate it and wire the
    ctypes-based hook from trn_agent_boot so neuron-profile exec times work.
    """
    import sys
    import types

    name = "antenv.axon_hooks"
    if name in sys.modules:
        return
    try:
        import antenv
        from trn_agent_boot.trn_boot import _ntff_profile_via_ctypes
    except ImportError:
        return
    hooks = types.ModuleType(name)
    hooks._the_hook = None
    hooks.set_axon_ntff_profile_hook = lambda h: setattr(hooks, "_the_hook", h)
    hooks.get_axon_ntff_profile_hook = lambda: hooks._the_hook
    sys.modules[name] = hooks
    antenv.axon_hooks = hooks
    so = "/opt/axon/libaxon_pjrt.so"
    if os.path.exists(so):
        hook = _ntff_profile_via_ctypes(so)
        if hook is not None:
            hooks.set_axon_ntff_profile_hook(hook)


def build_kernel() -> bass.Bass:
    nc = bacc.Bacc("TRN2", target_bir_lowering=False, debug=False, num_devices=8)

    # ---- DRAM I/O (per-core shard; weights replicated) ----
    x_d = nc.dram_tensor("x", (128, 2, 1024), BF16, kind="ExternalInput")
    wk_d = nc.dram_tensor("wk", (128, 2, 256), BF16, kind="ExternalInput")
    wq_d = nc.dram_tensor("wq", (128, 2, 256), BF16, kind="ExternalInput")
    wv_d = nc.dram_tensor("wv", (128, 2, 256), BF16, kind="ExternalInput")
    wpt_d = nc.dram_tensor("wpt", (128, 2, 256), BF16, kind="ExternalInput")
    bk_d = nc.dram_tensor("bk", (128, 2), F32, kind="ExternalInput")
    bq_d = nc.dram_tensor("bq", (128, 2), F32, kind="ExternalInput")
    bv_d = nc.dram_tensor("bv", (128, 2), F32, kind="ExternalInput")
    bvrow_d = nc.dram_tensor("bvrow", (1, 256), F32, kind="ExternalInput")
    wpe_d = nc.dram_tensor("wpe", (128, 2, 3), F32, kind="ExternalInput")
    bpe_d = nc.dram_tensor("bpe", (128, 2), F32, kind="ExternalInput")
    bproj_d = nc.dram_tensor("bproj", (128, 2), F32, kind="ExternalInput")
    onesb_d = nc.dram_tensor("onesb", (128, 1), BF16, kind="ExternalInput")
    onesf_d = nc.dram_tensor("onesf", (128, 32), F32, kind="ExternalInput")
    out_d = nc.dram_tensor("out", (128, 2, 1024), F32, kind="ExternalOutput")

    with tile.TileContext(nc) as tc:
        with (
            tc.tile_pool(name="const", bufs=1) as cpool,
            tc.tile_pool(name="work", bufs=4) as wpool,
            tc.tile_pool(name="epool", bufs=3) as epool,
            tc.tile_pool(name="ps_s", bufs=1, space="PSUM") as ps_s,
            tc.tile_pool(name="ps_av", bufs=1, space="PSUM") as ps_av,
            tc.tile_pool(name="ps_d", bufs=1, space="PSUM") as ps_d,
            tc.tile_pool(name="ps_misc", bufs=2, space="PSUM") as ps_misc,
        ):
            # ---- load constants / activations ----
            x_sb = cpool.tile([128, 2, 1024], BF16, tag="x")
            wk = cpool.tile([128, 2, 256], BF16, tag="wk")
            wq = cpool.tile([128, 2, 256], BF16, tag="wq")
            wv = cpool.tile([128, 2, 256], BF16, tag="wv")
            wpt = cpool.tile([128, 2, 256], BF16, tag="wpt")
            bk = cpool.tile([128, 2], F32, tag="bk")
            bq = cpool.tile([128, 2], F32, tag="bq")
            bv = cpool.tile([128, 2], F32, tag="bv")
            bvrow = cpool.tile([1, 256], F32, tag="bvrow")
            wpe = cpool.tile([128, 2, 3], F32, tag="wpe")
            bpe = cpool.tile([128, 2], F32, tag="bpe")
            bproj = cpool.tile([128, 2], F32, tag="bproj")
            onesb = cpool.tile([128, 1], BF16, tag="onesb")
            onesf = cpool.tile([128, 32], F32, tag="onesf")

            for sb, d in [
                (x_sb, x_d), (wk, wk_d), (wq, wq_d), (wv, wv_d), (wpt, wpt_d),
                (bk, bk_d), (bq, bq_d), (bv, bv_d), (bvrow, bvrow_d), (wpe, wpe_d),
                (bpe, bpe_d), (bproj, bproj_d), (onesb, onesb_d), (onesf, onesf_d),
            ]:
                nc.sync.dma_start(sb[:], d.ap())

            # persistent intermediates
            tk = cpool.tile([128, 2, 1024], BF16, tag="tk")        # packed k
            tq = cpool.tile([128, 2, 1024], BF16, tag="tq")        # packed q
            vnat = cpool.tile([128, 2, 1024], BF16, tag="vnat")    # v, natural
            vT = cpool.tile([128, 8, 256], BF16, tag="vT")         # v^T, j-chunked
            bvb = cpool.tile([128, 256], F32, tag="bvb")           # b_v row-broadcast
            peacc = cpool.tile([128, 2, 1024], BF16, tag="peacc")  # pe conv terms
            ybf = cpool.tile([128, 2, 1024], BF16, tag="ybf")      # y = av*R + pe
            zout = cpool.tile([128, 2, 1024], F32, tag="zout")

            # b_v broadcast across partitions for the v^T bias (free-dim bias)
            nc.gpsimd.partition_broadcast(bvb[:], bvrow[:], channels=128)

            # ---- phase-1 building blocks (emitted piecemeal) ----
            kq_ready = set()
            vt_ready = set()
            vn_ready = set()

            def emit_kq(t, n, w_sb, b_sb, dst):
                kq_ready.add((id(dst), t, n))
                ps = misc_ps()
                for kc in range(2):
                    nc.tensor.matmul(
                        ps[:], w_sb[:, kc, t * 128:(t + 1) * 128],
                        x_sb[:, kc, n * 512:(n + 1) * 512],
                        start=(kc == 0), stop=(kc == 1),
                    )
                nc.vector.tensor_scalar(
                    dst[:, t, n * 512:(n + 1) * 512], ps[:],
                    b_sb[:, t:t + 1], None, ALU.add,
                )

            def emit_vnat(t, n):
                vn_ready.add((t, n))
                ps = misc_ps()
                for kc in range(2):
                    nc.tensor.matmul(
                        ps[:], wv[:, kc, t * 128:(t + 1) * 128],
                        x_sb[:, kc, n * 512:(n + 1) * 512],
                        start=(kc == 0), stop=(kc == 1),
                    )
                nc.vector.tensor_scalar(
                    vnat[:, t, n * 512:(n + 1) * 512], ps[:],
                    bv[:, t:t + 1], None, ALU.add,
                )

            def emit_vt(jc):
                vt_ready.add(jc)
                ps = misc_ps()
                for kc in range(2):
                    nc.tensor.matmul(
                        ps[:, :256], x_sb[:, kc, jc * 128:(jc + 1) * 128],
                        wv[:, kc, :],
                        start=(kc == 0), stop=(kc == 1),
                    )
                nc.vector.tensor_tensor(vT[:, jc, :], ps[:, :256], bvb[:], ALU.add)

            # pe = depthwise conv(k=3, pad 1) on v + bias, into peacc[:, t, :]
            peacc_done = [False, False]
            pe_pending = []

            def emit_peacc(t):
                assert (t, 0) in vn_ready and (t, 1) in vn_ready
                nc.vector.tensor_scalar(
                    peacc[:, t, :], vnat[:, t, :], wpe[:, t, 1:2], bpe[:, t:t + 1],
                    ALU.mult, ALU.add,
                )
                tmp_l = wpool.tile([128, 1024], BF16, tag="pel", name=f"pel{t}")
                nc.vector.tensor_scalar(
                    tmp_l[:, :1023], vnat[:, t, :1023], wpe[:, t, 0:1], None,
                    ALU.mult,
                )
                nc.vector.tensor_tensor(
                    peacc[:, t, 1:], peacc[:, t, 1:], tmp_l[:, :1023], ALU.add,
                )
                tmp_r = wpool.tile([128, 1024], BF16, tag="per", name=f"per{t}")
                nc.vector.tensor_scalar(
                    tmp_r[:, :1023], vnat[:, t, 1:], wpe[:, t, 2:3], None,
                    ALU.mult,
                )
                nc.vector.tensor_tensor(
                    peacc[:, t, :1023], peacc[:, t, :1023], tmp_r[:, :1023], ALU.add,
                )
                peacc_done[t] = True
                for (tt_, nn_) in [p for p in pe_pending if p[0] == t]:
                    pe_pending.remove((tt_, nn_))
                    emit_pe_add(tt_, nn_)

            def emit_pe_add(t, n):
                nc.vector.tensor_tensor(
                    ybf[:, t, n * 512:(n + 1) * 512],
                    ybf[:, t, n * 512:(n + 1) * 512],
                    peacc[:, t, n * 512:(n + 1) * 512], ALU.add,
                )

            def emit_proj(mo, n, ps, bias_engine=None):
                for kc in range(2):
                    nc.tensor.matmul(
                        ps[:], wpt[:, kc, mo * 128:(mo + 1) * 128],
                        ybf[:, kc, n * 512:(n + 1) * 512],
                        start=(kc == 0), stop=(kc == 1),
                    )
                if bias_engine is nc.scalar:
                    nc.scalar.activation(
                        zout[:, mo, n * 512:(n + 1) * 512], ps[:],
                        AF.Identity, bias=bproj[:, mo:mo + 1],
                    )
                else:
                    nc.vector.tensor_scalar(
                        zout[:, mo, n * 512:(n + 1) * 512], ps[:],
                        bproj[:, mo:mo + 1], None, ALU.add,
                    )
                nc.sync.dma_start(
                    out_d.ap()[:, mo, n * 512:(n + 1) * 512],
                    zout[:, mo, n * 512:(n + 1) * 512],
                )

            # ---- phase 2: software-pipelined attention ----
            # steps (t, n, jc): per step one S quad / two exps / AV + d quads.
            # Phase-1 work not needed up front is drip-fed one chunk per step
            # ("extras") so ScalarE starts its exp chain as early as possible.
            steps = [
                (t, n, jc)
                for t in range(2) for n in range(2) for jc in range(8)
            ]
            av_tiles = {}
            d_tiles = {}

            def emit_s_step(step):
                # 4 score matmuls, one per row group -> run concurrently
                t, n, jc = step
                tiles = []
                assert (id(tk), t, 0) in kq_ready and (id(tq), t, n) in kq_ready
                assert jc < 4 or (id(tk), t, 1) in kq_ready
                for p in range(2):
                    s_ps = ps_s.tile(
                        [128, 1024], F32, tag="S", name=f"s_{t}_{n}_{jc}_{p}"
                    )
                    for gg in range(2):
                        g = 2 * p + gg
                        nc.tensor.matmul(
                            s_ps[:, gg * 512:(gg + 1) * 512],
                            tk[32 * g:32 * g + 16, t, jc * 128:(jc + 1) * 128],
                            tq[32 * g:32 * g + 16, t, n * 512:(n + 1) * 512],
                            start=True, stop=True,
                            tile_position=(32 * g, 0),
                        )
                    tiles.append(s_ps)
                return tiles

            def finish_tn(t, n):
                # R = 1/d (d ~ L, far from reciprocal_approx edge cases)
                av_ps = av_tiles.pop((t, n))
                d_ps = d_tiles.pop((t, n))
                rb_sb = wpool.tile([128, 512], F32, tag="rb")
                nc.vector.reciprocal_approx_fast(rb_sb[:], d_ps[:])
                nc.vector.tensor_tensor(
                    ybf[:, t, n * 512:(n + 1) * 512], av_ps[:], rb_sb[:],
                    ALU.mult,
                )
                if peacc_done[t]:
                    emit_pe_add(t, n)
                else:
                    pe_pending.append((t, n))

            # minimal front: q/k + first v^T chunk for step (0, 0, 0)
            emit_kq(0, 0, wk, bk, tk)
            emit_kq(0, 0, wq, bq, tq)
            emit_vt(0)
            # drip-fed producers; ORDER MATTERS: vT[jc] must be emitted no
            # later than step jc (its AV consumer), tk/tq halves before the
            # S quads that read them. Steps 1-2 pop two extras to make the
            # deadlines; readiness is assert-checked at the consumer sites.
            extras = {
                1: [lambda: emit_vt(1), lambda: emit_kq(0, 1, wk, bk, tk)],
                2: [lambda: emit_vt(2)],
                3: [lambda: emit_vt(3)],
                4: [lambda: emit_vt(4), lambda: emit_kq(0, 1, wq, bq, tq)],
                5: [lambda: emit_vt(5)],
                6: [lambda: emit_vt(6)],
                7: [lambda: emit_vt(7)],
                8: [lambda: emit_kq(1, 0, wk, bk, tk)],
                9: [lambda: emit_kq(1, 1, wk, bk, tk)],
                10: [lambda: emit_kq(1, 0, wq, bq, tq)],
                11: [lambda: emit_kq(1, 1, wq, bq, tq)],
                12: [lambda: emit_vnat(0, 0)],
                13: [lambda: emit_vnat(0, 1)],
                14: [lambda: emit_peacc(0)],
                15: [lambda: emit_vnat(1, 0)],
                16: [lambda: emit_vnat(1, 1)],
                17: [lambda: emit_peacc(1)],
                # n=0 proj units: both n=0 halves of y are ready after step
                # 23; run them here, borrowing the d-pool PSUM slot (its
                # (1,1) accumulation quads simply queue behind)
                26: [lambda: emit_proj(0, 0, ps_d.tile(
                    [128, 512], F32, tag="d", name="prj00"))],
                27: [lambda: emit_proj(1, 0, ps_d.tile(
                    [128, 512], F32, tag="d", name="prj10"))],
            }

            s_next = emit_s_step(steps[0])
            for i, step in enumerate(steps):
                t, n, jc = step
                s_cur = s_next
                e_sb = []
                for p in range(2):
                    e = epool.tile([128, 1024], BF16, tag="E", name=f"e{i}_{p}")
                    nc.scalar.activation(e[:], s_cur[p][:], AF.Exp, scale=SCALE)
                    e_sb.append(e)
                if i + 1 < len(steps):
                    s_next = emit_s_step(steps[i + 1])
                for fn in extras.pop(i, []):
                    fn()
                if (t, n) not in av_tiles:
                    av_tiles[(t, n)] = ps_av.tile(
                        [128, 512], F32, tag="av", name=f"av_{t}_{n}"
                    )
                    d_tiles[(t, n)] = ps_d.tile(
                        [128, 512], F32, tag="d", name=f"d_{t}_{n}"
                    )
                av_ps = av_tiles[(t, n)]
                d_ps = d_tiles[(t, n)]
                assert jc in vt_ready, (t, n, jc)
                for g in range(4):
                    # denominator, pre-broadcast: ones(128,32)^T @ E fills
                    # all 32 partitions of head g with d_h[i]. Emitted before
                    # the AV quad so the tail's reciprocal starts earlier.
                    nc.tensor.matmul(
                        d_ps[32 * g:32 * g + 32, :],
                        onesb[:, :32],
                        e_sb[g // 2][:, (g % 2) * 512:(g % 2 + 1) * 512],
                        start=(jc == 0), stop=(jc == 7),
                        tile_position=(0, 32 * g),
                        skip_group_check=True,
                    )
                for g in range(4):
                    h = 4 * t + g
                    nc.tensor.matmul(
                        av_ps[32 * g:32 * g + 32, :],
                        vT[:, jc, 32 * h:32 * h + 32],
                        e_sb[g // 2][:, (g % 2) * 512:(g % 2 + 1) * 512],
                        start=(jc == 0), stop=(jc == 7),
                        tile_position=(0, 32 * g),
                        skip_group_check=True,
                    )
                if jc == 7:
                    finish_tn(t, n)
            assert not extras

            # ---- phase 3: remaining (n=1) proj units ----
            # kc=0 (reads the long-finished t=0 half of y) is emitted first so
            # it executes off the critical tail chain; kc=1 + bias + DMA wait
            # only on the last pe add.
            prj_ps = []
            for mo in range(2):
                ps = misc_ps()
                prj_ps.append(ps)
                nc.tensor.matmul(
                    ps[:], wpt[:, 0, mo * 128:(mo + 1) * 128],
                    ybf[:, 0, 512:], start=True, stop=False,
                )
            for mo in range(2):
                ps = prj_ps[mo]
                nc.tensor.matmul(
                    ps[:], wpt[:, 1, mo * 128:(mo + 1) * 128],
                    ybf[:, 1, 512:], start=False, stop=True,
                )
                nc.scalar.activation(
                    zout[:, mo, 512:], ps[:],
                    AF.Identity, bias=bproj[:, mo:mo + 1],
                )
                nc.sync.dma_start(
                    out_d.ap()[:, mo, 512:], zout[:, mo, 512:],
                )

    nc.compile()
    return nc


def pack_inputs(x, w_qkv, b_qkv, w_pe, b_pe, w_proj, b_proj):
    """Host-side packing of the full inputs into per-core in_maps."""
    bf16 = ml_dtypes.bfloat16
    f32 = np.float32

    # k/q packed layouts: tile t in {0,1}; partition m = 32*g + r; head h = 4t+g.
    # Only r < 16 is live (k channel r -> qkv row 64h+16+r; q channel r -> 64h+r);
    # r >= 16 columns are zero so both tiles stay 32-aligned per head.
    w_kA = np.zeros((256, 256), dtype=w_qkv.dtype)
    w_qA = np.zeros((256, 256), dtype=w_qkv.dtype)
    b_kP = np.zeros((128, 2), dtype=b_qkv.dtype)
    b_qP = np.zeros((128, 2), dtype=b_qkv.dtype)
    for t in range(2):
        for m in range(128):
            g, r = m // 32, m % 32
            h = 4 * t + g
            if r < 16:
                w_kA[:, t * 128 + m] = w_qkv[64 * h + 16 + r]
                w_qA[:, t * 128 + m] = w_qkv[64 * h + r]
                b_kP[m, t] = b_qkv[64 * h + 16 + r]
                b_qP[m, t] = b_qkv[64 * h + r]

    v_rows = np.array([64 * (c // 32) + 32 + c % 32 for c in range(256)])
    w_v = w_qkv[v_rows].T  # (256 d, 256 c)
    b_v = b_qkv[v_rows]

    def kpart(a):  # (256, F) -> (128, 2, F)
        return np.ascontiguousarray(a.reshape(2, 128, -1).transpose(1, 0, 2))

    def chan2(a):  # (256,) -> (128, 2)
        return np.ascontiguousarray(a.reshape(2, 128).T)

    common = {
        "wk": kpart(w_kA).astype(bf16),
        "wq": kpart(w_qA).astype(bf16),
        "wv": kpart(w_v).astype(bf16),
        "wpt": kpart(w_proj.T).astype(bf16),
        "bk": b_kP.astype(f32),
        "bq": b_qP.astype(f32),
        "bv": chan2(b_v).astype(f32),
        "bvrow": np.ascontiguousarray(b_v[None, :]).astype(f32),
        "wpe": kpart(w_pe[:, 0, :]).astype(f32),
        "bpe": chan2(b_pe).astype(f32),
        "bproj": chan2(b_proj).astype(f32),
        "onesb": np.ones((128, 1), dtype=bf16),
        "onesf": np.ones((128, 32), dtype=f32),
    }
    in_maps = []
    for b in range(B):
        m = dict(common)
        m["x"] = kpart(x[b]).astype(bf16)
        in_maps.append(m)
    return in_maps


_CACHE = {}


def kernel(x, w_qkv, b_qkv, w_pe, b_pe, w_proj, b_proj):
    x = np.asarray(x, dtype=np.float32)
    w_qkv = np.asarray(w_qkv, dtype=np.float32)
    b_qkv = np.asarray(b_qkv, dtype=np.float32)
    w_pe = np.asarray(w_pe, dtype=np.float32)
    b_pe = np.asarray(b_pe, dtype=np.float32)
    w_proj = np.asarray(w_proj, dtype=np.float32)
    b_proj = np.asarray(b_proj, dtype=np.float32)

    if "nc" not in _CACHE:
        _CACHE["nc"] = build_kernel()
    nc = _CACHE["nc"]

    in_maps = pack_inputs(x, w_qkv, b_qkv, w_pe, b_pe, w_proj, b_proj)

    trace = os.environ.get("BASS_KERNEL_TRACE", "") == "1"
    if trace:
        _install_ntff_shim()
    res = run_bass_kernel_spmd(
        nc, in_maps, core_ids=list(range(B)), trace=trace,
    )
    if trace:
        _CACHE["last_result"] = res

    out = np.empty((B, DIM, L), dtype=np.float32)
    for b in range(B):
        z = res.results[b]["out"]  # (128, 2, 1024)
        out[b] = z.transpose(1, 0, 2).reshape(DIM, L)
    return out
